# revision 4
# baseline (speedup 1.0000x reference)
"""Trainium2 Bass kernel for masked attention softmax (ragged sequences).

Reference computation (per batch b):
    qp[k]   = sum_q query[b,0,q] * w[k,q]
    att[s]  = sum_k qp[k] * keys[b,s,k]
    score   = where(s < seq_len[b], att, NEG_INF)
    out[b]  = softmax(score)            # over s axis

Strategy (v2, fp16 fast-path):
  - Data-parallel over batch across 8 cores (512 batches/core, 4 tiles of 128).
  - Ragged trick: sort batches by seq_len descending (host-side), deal
    round-robin to cores so tile slot j has the same max length on every
    core; bake that extent into the kernel and only load/compute
    keys[:, :s_ext_j, :].
  - Mask fused into the data: host appends a mask element per (b,s)
    holding 0 (valid) or -1000 (masked); qp gets a fixed 1.0 appended, so
    the dot product includes the mask penalty (exp(-1000+x) == 0 in f32).
  - fp16 data path: keys shipped as float16 (halves HBM traffic vs f32)
    padded to KDA=130 elems so every per-position row starts 4B-aligned
    (DVE fast modes require packed 2-byte data at 4B-aligned addresses).
  - Per 128-batch tile (batch on partitions):
      * qp via one PE matmul (f32), converted to fp16 on ACT
      * score via TWO DVE passes instead of the 1x scalar_tensor_tensor
        (STT supports no DVE perf modes; tensor_tensor runs 2x on fp16 and
        tensor_scalar runs 4x):
          - per chunk: prod = kt * qp  (tensor_tensor, 2x, qp broadcast
            via a stride-0 AP -- no materialized replication)
          - per position: tensor_scalar(prod[:,s,:], 1.0, accum_out=att)
            at 4x -> 58+130/4 cyc ~ 94ns/position vs 207ns for fused STT
      * softmax without max-subtraction (|att| <= ~60; shift-invariant;
        seq_len==0 rows give 0/0 and are overwritten by the host):
        ACT exp(accum_out=sum) -> DVE reciprocal -> ACT copy(scale=1/sum)
  - Keys streamed in chunks (HWDGE), geometric ramp-up on the first
    tile so the DVE starts early.
  - Host scatters per-core outputs back via inverse permutation; rows with
    seq_len == 0 are uniform 1/S (reference softmax of all-equal scores).

  Numerics: fp16 keys/qp give per-element relative error ~2.4e-4; the dot
  product error is ~5e-3 absolute in att; softmax output error lands
  ~1e-3..4e-3 -- well under the 2e-2 gate (bf16, 8x coarser, measured
  3.3e-2).
"""

import sys

import numpy as np

sys.path.insert(0, "/opt/trn_rl_repo")

import concourse.bass as bass
import concourse.tile as tile
from concourse import bacc, mybir
from concourse.bass_utils import run_bass_kernel_spmd


def _install_trace_shims():
    """The agent image lacks ``antenv.axon_hooks``, so trace=True silently
    degrades.  Recreate the module and register the ctypes NTFF hook from
    trn_agent_boot; also make artifact upload failure non-fatal."""
    try:
        import types

        import antenv
        from concourse import bass_utils as _bu

        if "antenv.axon_hooks" not in sys.modules:
            mod = types.ModuleType("antenv.axon_hooks")
            mod._hook = None
            mod.set_axon_ntff_profile_hook = lambda h: setattr(mod, "_hook", h)
            mod.get_axon_ntff_profile_hook = lambda: mod._hook
            sys.modules["antenv.axon_hooks"] = mod
            antenv.axon_hooks = mod
            from trn_agent_boot.trn_boot import _ntff_profile_via_ctypes

            mod.set_axon_ntff_profile_hook(
                _ntff_profile_via_ctypes("/opt/axon/libaxon_pjrt.so")
            )

        _orig_upload = _bu.upload_artifacts

        def _safe_upload(tmpdir):
            try:
                return _orig_upload(tmpdir)
            except Exception:
                return "local://" + str(tmpdir)

        _bu.upload_artifacts = _safe_upload
    except Exception:
        pass


_install_trace_shims()

B, S, KD, QD = 4096, 200, 128, 128
NCORES = 8
P = 128
PB = B // NCORES           # batches per core
NTILES = PB // P           # partition tiles per core
NEG_PEN = -1000.0          # mask penalty: exp(-1000+x) == 0 in f32
CH = 50                    # s-positions per keys DMA chunk
KDA = KD + 2               # keys padded: [0:128]=keys, [128]=mask, [129]=0
                           # (even KDA keeps rows 4B-aligned for DVE 2x/4x)

LAST_RESULTS = None
_nc_cache = {}


def _build(s_exts):
    f16 = mybir.dt.float16
    f32 = mybir.dt.float32
    nc = bacc.Bacc("TRN2", target_bir_lowering=False, debug=False)
    keys_d = nc.dram_tensor("keys", [PB, S, KDA], f16, kind="ExternalInput")
    # qw[j] = [qT_j | wT] fused so each tile's matmul depends on ONE dma
    qw_d = nc.dram_tensor("qw", [QD, NTILES, P + KD], f32, kind="ExternalInput")
    out_d = nc.dram_tensor("out", [PB, S], f32, kind="ExternalOutput")

    with tile.TileContext(nc) as tc:
        with (
            tc.tile_pool(name="keys", bufs=4) as keysp,
            tc.tile_pool(name="prod", bufs=3) as prodp,
            tc.tile_pool(name="small", bufs=2) as smallp,
            tc.tile_pool(name="qpp", bufs=NTILES) as qpp,
            tc.tile_pool(name="scr", bufs=8) as scrp,
            tc.tile_pool(name="psum", bufs=4, space=bass.MemorySpace.PSUM) as psump,
        ):
            # qp for ALL tiles up-front via ONE fused qw DMA; PE/ACT are
            # otherwise idle, so every tile's qp is ready long before its
            # first chunk multiply.
            qw = smallp.tile([QD, NTILES, P + KD], f32, tag="qw")
            nc.sync.dma_start(qw[:], qw_d[:])
            qps = []
            for j in range(NTILES):
                # qp[b,k] = sum_q qT[q,b] * wT[q,k]; qp[:,128:130] = 1.0 so
                # the mask element contributes the penalty.
                qp_ps = psump.tile([P, KD], f32, tag="qp_ps")
                nc.tensor.matmul(
                    qp_ps[:], qw[:, j, :P], qw[:, j, P : P + KD],
                    start=True, stop=True,
                )
                qp = qpp.tile([P, KDA], f16, tag=f"qp{j}")
                nc.gpsimd.memset(qp[:, KD:KDA], 1.0)
                nc.scalar.copy(qp[:, :KD], qp_ps[:])  # f32 -> f16 on ACT
                qps.append(qp)

            kt0 = keysp.tile([P, CH, KDA], f16, tag="kt")
            nc.sync.dma_start(kt0[:, :8, :], keys_d[0:P, 0:8, :])

            for j in range(NTILES):
                E = s_exts[j]
                qp = qps[j]

                # chunk schedule: geometric ramp on tile 0 so DVE starts
                # as soon as the first keys land and never starves early.
                chunks = []
                c0 = 0
                if j == 0:
                    for ch in (8, 16, 26):
                        chunks.append((c0, ch))
                        c0 += ch
                while c0 < E:
                    ch = min(CH, E - c0)
                    chunks.append((c0, ch))
                    c0 += ch

                att = smallp.tile([P, E], f32, tag="att")
                for c0, ch in chunks:
                    if j == 0 and c0 == 0:
                        kt = kt0  # prefetched above
                    else:
                        kt = keysp.tile([P, CH, KDA], f16, tag="kt")
                        nc.sync.dma_start(
                            kt[:, :ch, :],
                            keys_d[j * P : (j + 1) * P, c0 : c0 + ch, :],
                        )
                    # prod = kt * qp (broadcast along s): fp16 SBUF packed
                    # -> DVE 2x mode, one instruction per chunk.
                    prod = prodp.tile([P, CH, KDA], f16, tag="prod")
                    qp_b = qp[:].unsqueeze(1).broadcast_to([P, ch, KDA])
                    nc.vector.tensor_tensor(
                        prod[:, :ch, :], kt[:, :ch, :], qp_b,
                        op=mybir.AluOpType.mult,
                    )
                    for s in range(ch):
                        # masked score: accum_out = sum_k prod[:, s, k]
                        # tensor_scalar is single-src -> DVE 4x mode.
                        scr = scrp.tile([P, KDA], f16, tag="scr")
                        nc.vector.tensor_scalar(
                            scr[:],
                            prod[:, s, :],
                            1.0,
                            0.0,
                            op0=mybir.AluOpType.mult,
                            op1=mybir.AluOpType.add,
                            accum_out=att[:, c0 + s : c0 + s + 1],
                        )

                # softmax without max-subtraction (|att| <= ~60 here;
                # seq_len==0 rows give 0/0 but the host overwrites them).
                e_t = smallp.tile([P, E], f32, tag="e")
                ssum = smallp.tile([P, 1], f32, tag="ssum")
                nc.scalar.activation(
                    e_t[:],
                    att[:],
                    mybir.ActivationFunctionType.Exp,
                    bias=0.0,
                    scale=1.0,
                    accum_out=ssum[:],
                )
                rec = smallp.tile([P, 1], f32, tag="rec")
                nc.vector.reciprocal(rec[:], ssum[:])
                o_t = smallp.tile([P, E], f32, tag="o")
                # final scale on the (otherwise idle) ACT engine
                nc.scalar.mul(o_t[:], e_t[:], rec[:])
                # out via SWDGE (gpsimd) so the Sync queue carries only
                # keys chunks -- a keys issue never waits behind an out issue
                nc.gpsimd.dma_start(out_d[j * P : (j + 1) * P, 0:E], o_t[:])
    nc.compile()
    return nc


def _prep(query, keys, seq_len, w):
    query = np.ascontiguousarray(np.asarray(query), dtype=np.float32)
    keys = np.asarray(keys)
    w = np.ascontiguousarray(np.asarray(w), dtype=np.float32)
    lens = np.asarray(seq_len).reshape(B).astype(np.int64)

    order = np.argsort(-lens, kind="stable")
    gp = NCORES * P  # batches per tile slot across all cores
    slot_max = [int(lens[order[j * gp : (j + 1) * gp]].max()) for j in range(NTILES)]
    s_exts = tuple(min(S, max(1, m)) for m in slot_max)

    perms = []
    for c in range(NCORES):
        perms.append(
            np.concatenate(
                [order[j * gp : (j + 1) * gp][c::NCORES] for j in range(NTILES)]
            )
        )

    keys16 = keys.astype(np.float16)
    wT = np.ascontiguousarray(w.T)
    arange_s = np.arange(S, dtype=np.int64)[None, :]
    in_maps = []
    for c in range(NCORES):
        pc = perms[c]
        qT = query[pc, 0, :].reshape(NTILES, P, QD).transpose(2, 0, 1)
        qw = np.empty((QD, NTILES, P + KD), dtype=np.float32)
        qw[:, :, :P] = qT
        qw[:, :, P:] = wT[:, None, :]
        keys_aug = np.empty((PB, S, KDA), dtype=np.float16)
        keys_aug[:, :, :KD] = keys16[pc]
        keys_aug[:, :, KD] = np.where(
            arange_s < lens[pc][:, None], np.float16(0.0), np.float16(NEG_PEN)
        )
        keys_aug[:, :, KD + 1] = np.float16(0.0)
        in_maps.append({"keys": keys_aug, "qw": qw})
    return lens, s_exts, perms, in_maps


def kernel(query, keys, seq_len, w):
    global LAST_RESULTS
    lens, s_exts, perms, in_maps = _prep(query, keys, seq_len, w)

    nc = _nc_cache.get(s_exts)
    if nc is None:
        nc = _build(s_exts)
        _nc_cache[s_exts] = nc

    res = run_bass_kernel_spmd(nc, in_maps, core_ids=list(range(NCORES)))
    LAST_RESULTS = res

    out = np.zeros((B, S), dtype=np.float32)
    for c in range(NCORES):
        dev = np.asarray(res.results[c]["out"])
        pc = perms[c]
        for j in range(NTILES):
            E = s_exts[j]
            rows = pc[j * P : (j + 1) * P]
            out[rows, :E] = dev[j * P : (j + 1) * P, :E]
    out[lens == 0, :] = np.float32(1.0 / S)
    return out


# revision 5
# speedup vs baseline: 1.8271x; 1.8271x over previous
"""Trainium2 Bass kernel for masked attention softmax (ragged sequences).

Reference computation (per batch b):
    qp[k]   = sum_q query[b,0,q] * w[k,q]
    att[s]  = sum_k qp[k] * keys[b,s,k]
    score   = where(s < seq_len[b], att, NEG_INF)
    out[b]  = softmax(score)            # over s axis

Strategy (v3, fp16 2x tree):
  - Data-parallel over batch across 8 cores (512 batches/core, 4 tiles of 128).
  - Ragged trick: sort batches by seq_len descending (host-side), deal
    round-robin to cores so tile slot j has the same max length on every
    core; bake that extent into the kernel and only load/compute
    keys[:, :s_ext_j, :].
  - Mask fused into the data: host appends a mask element per (b,s)
    holding 0 (valid) or -1000 (masked); qp gets a fixed 1.0 appended, so
    the dot product includes the mask penalty (exp(-1000+x) == 0 in f32).
  - fp16 data path, KDA=130 (even -> every per-position row 4B-aligned,
    required for the DVE 2x mode: 2-byte dtype, packed, aligned, SBUF).
  - Measured op costs (DVE @0.96GHz, ~58cyc init + FD/mode cycles):
      scalar_tensor_tensor (+accum): no fast modes -> ~207ns/position
      tensor_scalar(+accum): 1x + accumulator drain -> ~330ns/position
      tensor_tensor fp16: 2x (the only fast 2-src op)
      tensor_reduce: 1x
    so the score is computed chunk-granular with TT 2x everywhere:
      * prod = kt * qp   (TT mult, qp broadcast via stride-0 AP)
      * tree: 128->64->32->16->8 halving TT adds (fp16 2x), leftover
        mask pair added into the front, then one tensor_reduce
        [P,ch,8] -> att[P,ch] (fp32)
    ~142ns/position vs 207ns for the baseline fused STT.
  - qp via one PE matmul per tile (f32), converted to fp16 on ACT.
  - Softmax tail: ACT exp with accum_out gives e and row-sum; BOTH are
    DMA'd out and the final divide happens on the host during unshard
    (removes the DVE reciprocal whose semaphore stalled the DVE queue
    ~19us/tile in the baseline trace).
  - Keys streamed in chunks (HWDGE), geometric ramp-up on the first tile.
  - Host scatters per-core outputs back via inverse permutation; rows with
    seq_len == 0 are uniform 1/S.
"""

import sys

import numpy as np

sys.path.insert(0, "/opt/trn_rl_repo")

import concourse.bass as bass
import concourse.tile as tile
from concourse import bacc, mybir
from concourse.bass_utils import run_bass_kernel_spmd


def _install_trace_shims():
    """The agent image lacks ``antenv.axon_hooks``, so trace=True silently
    degrades.  Recreate the module and register the ctypes NTFF hook from
    trn_agent_boot; also make artifact upload failure non-fatal."""
    try:
        import types

        import antenv
        from concourse import bass_utils as _bu

        if "antenv.axon_hooks" not in sys.modules:
            mod = types.ModuleType("antenv.axon_hooks")
            mod._hook = None
            mod.set_axon_ntff_profile_hook = lambda h: setattr(mod, "_hook", h)
            mod.get_axon_ntff_profile_hook = lambda: mod._hook
            sys.modules["antenv.axon_hooks"] = mod
            antenv.axon_hooks = mod
            from trn_agent_boot.trn_boot import _ntff_profile_via_ctypes

            mod.set_axon_ntff_profile_hook(
                _ntff_profile_via_ctypes("/opt/axon/libaxon_pjrt.so")
            )

        _orig_upload = _bu.upload_artifacts

        def _safe_upload(tmpdir):
            try:
                return _orig_upload(tmpdir)
            except Exception:
                return "local://" + str(tmpdir)

        _bu.upload_artifacts = _safe_upload
    except Exception:
        pass


_install_trace_shims()

B, S, KD, QD = 4096, 200, 128, 128
NCORES = 8
P = 128
PB = B // NCORES           # batches per core
NTILES = PB // P           # partition tiles per core
NEG_PEN = -1000.0          # mask penalty: exp(-1000+x) == 0 in f32
CH = 50                    # s-positions per keys DMA chunk
KDA = KD + 2               # keys padded: [0:128]=keys, [128]=mask, [129]=0

LAST_RESULTS = None
_nc_cache = {}


def _build(s_exts):
    f16 = mybir.dt.float16
    f32 = mybir.dt.float32
    mult = mybir.AluOpType.mult
    add = mybir.AluOpType.add
    nc = bacc.Bacc("TRN2", target_bir_lowering=False, debug=False)
    keys_d = nc.dram_tensor("keys", [PB, S, KDA], f16, kind="ExternalInput")
    # qw[j] = [qT_j | wT] fused so each tile's matmul depends on ONE dma
    qw_d = nc.dram_tensor("qw", [QD, NTILES, P + KD], f32, kind="ExternalInput")
    e_d = nc.dram_tensor("e", [PB, S], f32, kind="ExternalOutput")
    ssum_d = nc.dram_tensor("ssum", [PB, 1], f32, kind="ExternalOutput")

    with tile.TileContext(nc) as tc:
        with (
            tc.tile_pool(name="keys", bufs=4) as keysp,
            tc.tile_pool(name="prod", bufs=2) as prodp,
            tc.tile_pool(name="tree", bufs=2) as treep,
            tc.tile_pool(name="small", bufs=2) as smallp,
            tc.tile_pool(name="qpp", bufs=NTILES) as qpp,
            tc.tile_pool(name="psum", bufs=4, space=bass.MemorySpace.PSUM) as psump,
        ):
            # qp for ALL tiles up-front via ONE fused qw DMA; PE/ACT are
            # otherwise idle, so every tile's qp is ready long before its
            # first chunk multiply.
            qw = smallp.tile([QD, NTILES, P + KD], f32, tag="qw")
            nc.sync.dma_start(qw[:], qw_d[:])
            qps = []
            for j in range(NTILES):
                # qp[b,k] = sum_q qT[q,b] * wT[q,k]; qp[:,128:130] = 1.0 so
                # the mask element contributes the penalty.
                qp_ps = psump.tile([P, KD], f32, tag="qp_ps")
                nc.tensor.matmul(
                    qp_ps[:], qw[:, j, :P], qw[:, j, P : P + KD],
                    start=True, stop=True,
                )
                qp = qpp.tile([P, KDA], f16, tag=f"qp{j}")
                nc.gpsimd.memset(qp[:, KD:KDA], 1.0)
                nc.scalar.copy(qp[:, :KD], qp_ps[:])  # f32 -> f16 on ACT
                qps.append(qp)

            kt0 = keysp.tile([P, CH, KDA], f16, tag="kt")
            nc.sync.dma_start(kt0[:, :8, :], keys_d[0:P, 0:8, :])

            for j in range(NTILES):
                E = s_exts[j]
                qp = qps[j]

                # chunk schedule: geometric ramp on tile 0 so DVE starts
                # as soon as the first keys land and never starves early.
                chunks = []
                c0 = 0
                if j == 0:
                    for ch in (8, 16, 26):
                        chunks.append((c0, ch))
                        c0 += ch
                while c0 < E:
                    ch = min(CH, E - c0)
                    chunks.append((c0, ch))
                    c0 += ch

                att = smallp.tile([P, E], f32, tag="att")
                for c0, ch in chunks:
                    if j == 0 and c0 == 0:
                        kt = kt0  # prefetched above
                    else:
                        kt = keysp.tile([P, CH, KDA], f16, tag="kt")
                        nc.sync.dma_start(
                            kt[:, :ch, :],
                            keys_d[j * P : (j + 1) * P, c0 : c0 + ch, :],
                        )
                    # prod = kt * qp (broadcast along s): fp16 packed SBUF
                    # -> DVE 2x, one instruction per chunk.
                    prod = prodp.tile([P, CH, KDA], f16, tag="prod")
                    qp_b = qp[:].unsqueeze(1).broadcast_to([P, ch, KDA])
                    nc.vector.tensor_tensor(
                        prod[:, :ch, :], kt[:, :ch, :], qp_b, op=mult
                    )
                    # halving tree, all fp16 TT adds at 2x
                    r1 = treep.tile([P, CH, 64], f16, tag="r1")
                    nc.vector.tensor_tensor(
                        r1[:, :ch, :], prod[:, :ch, 0:64], prod[:, :ch, 64:128],
                        op=add,
                    )
                    r2 = treep.tile([P, CH, 32], f16, tag="r2")
                    nc.vector.tensor_tensor(
                        r2[:, :ch, :], r1[:, :ch, 0:32], r1[:, :ch, 32:64],
                        op=add,
                    )
                    r3 = treep.tile([P, CH, 16], f16, tag="r3")
                    nc.vector.tensor_tensor(
                        r3[:, :ch, :], r2[:, :ch, 0:16], r2[:, :ch, 16:32],
                        op=add,
                    )
                    r4 = treep.tile([P, CH, 8], f16, tag="r4")
                    nc.vector.tensor_tensor(
                        r4[:, :ch, :], r3[:, :ch, 0:8], r3[:, :ch, 8:16],
                        op=add,
                    )
                    # fold the [mask, 0] leftover pair into r4[:, :, 0:2]
                    nc.vector.tensor_tensor(
                        r4[:, :ch, 0:2], r4[:, :ch, 0:2], prod[:, :ch, 128:130],
                        op=add,
                    )
                    # final 8 -> 1 segmented reduce, fp32 out
                    nc.vector.tensor_reduce(
                        att[:, c0 : c0 + ch], r4[:, :ch, :],
                        axis=mybir.AxisListType.X, op=add,
                    )

                # exp with row-sum accumulation; the 1/sum divide happens
                # host-side during unshard (no DVE reciprocal stall).
                e_t = smallp.tile([P, E], f32, tag="e")
                ssum = smallp.tile([P, 1], f32, tag="ssum")
                nc.scalar.activation(
                    e_t[:],
                    att[:],
                    mybir.ActivationFunctionType.Exp,
                    bias=0.0,
                    scale=1.0,
                    accum_out=ssum[:],
                )
                # out via SWDGE (gpsimd) so the Sync queue carries only keys
                nc.gpsimd.dma_start(e_d[j * P : (j + 1) * P, 0:E], e_t[:])
                nc.gpsimd.dma_start(ssum_d[j * P : (j + 1) * P, :], ssum[:])
    nc.compile()
    return nc


def _prep(query, keys, seq_len, w):
    query = np.ascontiguousarray(np.asarray(query), dtype=np.float32)
    keys = np.asarray(keys)
    w = np.ascontiguousarray(np.asarray(w), dtype=np.float32)
    lens = np.asarray(seq_len).reshape(B).astype(np.int64)

    order = np.argsort(-lens, kind="stable")
    gp = NCORES * P  # batches per tile slot across all cores
    slot_max = [int(lens[order[j * gp : (j + 1) * gp]].max()) for j in range(NTILES)]
    s_exts = tuple(min(S, max(1, m)) for m in slot_max)

    perms = []
    for c in range(NCORES):
        perms.append(
            np.concatenate(
                [order[j * gp : (j + 1) * gp][c::NCORES] for j in range(NTILES)]
            )
        )

    keys16 = keys.astype(np.float16)
    wT = np.ascontiguousarray(w.T)
    arange_s = np.arange(S, dtype=np.int64)[None, :]
    in_maps = []
    for c in range(NCORES):
        pc = perms[c]
        qT = query[pc, 0, :].reshape(NTILES, P, QD).transpose(2, 0, 1)
        qw = np.empty((QD, NTILES, P + KD), dtype=np.float32)
        qw[:, :, :P] = qT
        qw[:, :, P:] = wT[:, None, :]
        keys_aug = np.empty((PB, S, KDA), dtype=np.float16)
        keys_aug[:, :, :KD] = keys16[pc]
        keys_aug[:, :, KD] = np.where(
            arange_s < lens[pc][:, None], np.float16(0.0), np.float16(NEG_PEN)
        )
        keys_aug[:, :, KD + 1] = np.float16(0.0)
        in_maps.append({"keys": keys_aug, "qw": qw})
    return lens, s_exts, perms, in_maps


def kernel(query, keys, seq_len, w):
    global LAST_RESULTS
    lens, s_exts, perms, in_maps = _prep(query, keys, seq_len, w)

    nc = _nc_cache.get(s_exts)
    if nc is None:
        nc = _build(s_exts)
        _nc_cache[s_exts] = nc

    res = run_bass_kernel_spmd(nc, in_maps, core_ids=list(range(NCORES)))
    LAST_RESULTS = res

    out = np.zeros((B, S), dtype=np.float32)
    for c in range(NCORES):
        e = np.asarray(res.results[c]["e"])
        ssum = np.asarray(res.results[c]["ssum"]).reshape(PB, 1)
        pc = perms[c]
        for j in range(NTILES):
            E = s_exts[j]
            rows = pc[j * P : (j + 1) * P]
            blk = e[j * P : (j + 1) * P, :E] / ssum[j * P : (j + 1) * P]
            out[rows, :E] = blk
    out[lens == 0, :] = np.float32(1.0 / S)
    return out
